# revision 1
# baseline (speedup 1.0000x reference)
"""Causal self-attention with RoPE on 8 Trainium2 NeuronCores.

Problem (hardcoded): x (4, 2048, 2048) f32, w_attn (2048, 6144),
w_proj (2048, 2048), rope_cos/rope_sin (2048, 64), 16 heads, hd=128.

Sharding: 8 cores = 4 batches x 2 head-groups (8 heads each).  Each core
computes qkv projection for its heads, RoPE, causal attention, and a
partial output projection (its head-group's rows of w_proj).  The host
sums the two partials per batch (the "all-reduce after c_proj") and
transposes back, since the device kernel works fully transposed.

Device layout choices:
  - qT, kT stored [hd=128 partitions, T free]; S^T tiles [j_keys, q]
    come straight from matmul(lhsT=kT_j, rhs=qT_q).  Softmax exp is
    elementwise (no max subtraction needed: scores ~ N(0,1), max ~ 6);
    causality = skipping j>q blocks + masking diagonal blocks.  The PV
    matmul consumes P^T directly with v in natural [T, hd] layout as
    lhsT, producing o^T with no transposes anywhere.
  - RoPE pairs (2i, 2i+1) are host-permuted to (i, 64+i) by permuting
    w_attn's q/k columns (dot products are permutation invariant), so
    the rotation acts on contiguous partition ranges.
  - All matmuls run in float32r (fp32 bits, full-rate PE mode,
    ~1.5e-4 scale-relative error measured on K=2048).
"""

import sys

sys.path.insert(0, "/opt/trn_rl_repo")

import numpy as np

import concourse.bass as bass
import concourse.mybir as mybir
import concourse.tile as tile

F32 = mybir.dt.float32
F32R = mybir.dt.float32r
P = 128


# --------------------------------------------------------------------------
# This container's walrus build rejects any instruction carrying more than
# one sem wait.  Split extras onto NoOps inserted before the instruction on
# the same engine (per-engine program order makes the waits complete first).
def _split_multi_waits(nc):
    n = 0
    for fn in nc.m.functions:
        for bb in fn.blocks:
            out = []
            changed = False
            for inst in bb.instructions:
                si = inst.sync_info
                waits = list(si.on_wait or []) if si is not None else []
                if len(waits) > 1:
                    changed = True
                    n += 1
                    for w in waits[:-1]:
                        nop = mybir.InstNoOp(
                            name=nc.get_next_instruction_name(),
                            engine=inst.engine,
                            ins=[],
                            outs=[],
                            sync_info=mybir.SyncInfo(on_wait=[w], on_update=[]),
                        )
                        try:
                            nc.register_instruction(nop, overwrite=True)
                        except Exception:
                            pass
                        out.append(nop)
                    inst.sync_info = mybir.SyncInfo(
                        on_wait=[waits[-1]], on_update=list(si.on_update or [])
                    )
                out.append(inst)
            if changed:
                bb.instructions = out
    return n


def _r(ap):
    return ap.bitcast(F32R)


def build_attention_core(T=2048, C=2048, G=8, n_half=2):
    """One core\'s program.  T tokens, C model dim, G heads in this core\'s
    group (hd=128 each).  Returns the Bass object."""
    KO = C // P          # contraction tiles over model dim
    TH = T // n_half     # tokens per phase-A pass
    NTC = max(TH // 512, 1)   # 512-wide t chunks per half (phase A qk)
    TCW = min(TH, 512)
    NTB = TH // P        # 128-tall t blocks per half (phase A v)
    VN = min(512, G * P)  # v column chunk
    NV = (G * P) // VN
    NQ = max(T // 512, 1)  # 512-wide q chunks (phase B)
    QW = min(T, 512)
    JPQ = QW // P        # j tiles per q chunk width
    NJ = T // P          # total j tiles
    KQ = max(KO // 4, 1)  # kc per xt quarter tile
    NXQ = KO // KQ

    nc = bass.Bass()
    xt = nc.dram_tensor("xt", [n_half, P, KO, TH], F32R, kind="ExternalInput")
    wqk = nc.dram_tensor("wqk", [2 * G, P, KO, P], F32R, kind="ExternalInput")
    wv = nc.dram_tensor("wv", [NV, P, KO, VN], F32R, kind="ExternalInput")
    wp = nc.dram_tensor("wp", [KO, P, G, P], F32R, kind="ExternalInput")
    # cosd = [cos; cos], sind = [-sin; +sin]  (rope = raw*cosd + swapped*sind)
    cosp = nc.dram_tensor("cosp", [P, T], F32, kind="ExternalInput")
    sinp = nc.dram_tensor("sinp", [P, T], F32, kind="ExternalInput")
    maskt = nc.dram_tensor("maskt", [P, P], F32, kind="ExternalInput")
    ones_s = nc.dram_tensor("ones_s", [P, 1], F32R, kind="ExternalInput")
    ones1 = nc.dram_tensor("ones1", [1, P], F32R, kind="ExternalInput")
    outT = nc.dram_tensor("outT", [C, T], F32, kind="ExternalOutput")

    scale = 1.0 / np.sqrt(128.0)

    with tile.TileContext(nc) as tc:
        with (
            tc.tile_pool(name="dram", bufs=1, space="DRAM") as dram,
            tc.tile_pool(name="const", bufs=1) as cpool,
        ):
            qkd = dram.tile([2 * G, P, T], F32R)
            od = dram.tile([G, P, T], F32R)

            cos_s = cpool.tile([P, T], F32)
            nc.sync.dma_start(cos_s[:], cosp[:])
            sin_s = cpool.tile([P, T], F32)
            nc.sync.dma_start(sin_s[:], sinp[:])
            mask_s = cpool.tile([P, P], F32)
            nc.sync.dma_start(mask_s[:], maskt[:])
            one_col = cpool.tile([P, 1], F32R)
            nc.sync.dma_start(one_col[:], ones_s[:])
            one_row = cpool.tile([1, P], F32R)
            nc.sync.dma_start(one_row[:], ones1[:])

            with tc.tile_pool(name="vall", bufs=1) as va_pool:
                # v stays resident in SBUF through phases A and B:
                # v_all[ti, to, hh*128+d] = v[to*128+ti, head hh, d]
                v_all = va_pool.tile([P, NJ, G * P], F32R, tag="vall")

                # ------------- Phase A: qkT + RoPE, v -------------
                with (
                    tc.tile_pool(name="xt", bufs=4) as xt_pool,
                    tc.tile_pool(name="wqk", bufs=2) as wqk_pool,
                    tc.tile_pool(name="wv", bufs=1) as wv_pool,
                    tc.tile_pool(name="qkraw", bufs=1) as qkraw_pool,
                    tc.tile_pool(name="roped", bufs=1) as roped_pool,
                    tc.tile_pool(name="ropetmp", bufs=1) as rtmp_pool,
                    tc.tile_pool(name="psA", bufs=2, space="PSUM") as psA,
                    tc.tile_pool(name="psV", bufs=2, space="PSUM") as psV,
                ):
                    for H in range(n_half):
                        t0 = H * TH
                        xtq = []
                        for qq in range(NXQ):
                            xq = xt_pool.tile([P, KQ, TH], F32R, tag="xtq",
                                              name=f"xtq{qq}")
                            nc.sync.dma_start(
                                xq[:], xt[H, :, qq * KQ : (qq + 1) * KQ, :]
                            )
                            xtq.append(xq)
                        # --- q,k heads ---
                        for m in range(2 * G):
                            w_s = wqk_pool.tile([P, KO, P], F32R, tag="wqk")
                            nc.sync.dma_start(w_s[:], wqk[m])
                            pss = [
                                psA.tile([P, TCW], F32, tag=f"pqk{i}",
                                         name=f"pqk{i}")
                                for i in range(NTC)
                            ]
                            for kc in range(KO):
                                for i in range(NTC):
                                    nc.tensor.matmul(
                                        pss[i][:],
                                        w_s[:, kc, :],
                                        xtq[kc // KQ][
                                            :, kc % KQ,
                                            i * TCW : (i + 1) * TCW,
                                        ],
                                        start=(kc == 0),
                                        stop=(kc == KO - 1),
                                    )
                            # RoPE: rope = raw*[cos;cos] + swap(raw)*[-sin;+sin]
                            raw = qkraw_pool.tile([P, TH], F32, tag="qkraw")
                            sw = rtmp_pool.tile([P, TH], F32, tag="rtmp")
                            rop = roped_pool.tile([P, TH], F32R, tag="roped")
                            for i in range(NTC):
                                sl = slice(i * TCW, (i + 1) * TCW)
                                nc.any.tensor_copy(raw[:, sl], pss[i][:])
                                nc.vector.tensor_mul(
                                    rop[:, sl], pss[i][:],
                                    cos_s[:, t0 + i * TCW : t0 + (i + 1) * TCW],
                                )
                            nc.sync.dma_start(sw[0:64, :], raw[64:128, :])
                            nc.sync.dma_start(sw[64:128, :], raw[0:64, :])
                            nc.vector.tensor_mul(
                                sw[:], sw[:], sin_s[:, t0 : t0 + TH]
                            )
                            nc.vector.tensor_add(rop[:], rop[:], sw[:])
                            nc.sync.dma_start(qkd[m, :, t0 : t0 + TH], rop[:])
                        # --- v (kept in SBUF, natural [t, d] layout) ---
                        for n2 in range(NV):
                            wv_s = wv_pool.tile([P, KO, VN], F32R, tag="wv")
                            nc.sync.dma_start(wv_s[:], wv[n2])
                            for tb in range(NTB):
                                psv = psV.tile([P, VN], F32, tag="pv")
                                for kc in range(KO):
                                    nc.tensor.matmul(
                                        psv[:],
                                        xtq[kc // KQ][
                                            :, kc % KQ, tb * P : (tb + 1) * P
                                        ],
                                        wv_s[:, kc, :],
                                        start=(kc == 0),
                                        stop=(kc == KO - 1),
                                    )
                                nc.any.tensor_copy(
                                    v_all[
                                        :, H * NTB + tb,
                                        n2 * VN : (n2 + 1) * VN,
                                    ],
                                    psv[:],
                                )

                # ------------- Phase B: attention per head -------------
                with (
                    tc.tile_pool(name="qh", bufs=3) as q_pool,
                    tc.tile_pool(name="kh", bufs=3) as k_pool,
                    tc.tile_pool(name="pt", bufs=6) as pt_pool,
                    tc.tile_pool(name="racc", bufs=2) as racc_pool,
                    tc.tile_pool(name="rsb", bufs=2) as rsb_pool,
                    tc.tile_pool(name="rinv", bufs=2) as rinv_pool,
                    tc.tile_pool(name="rq", bufs=2) as rq_pool,
                    tc.tile_pool(name="dramq", bufs=2, space="DRAM") as dramq,
                    tc.tile_pool(name="oacc", bufs=2) as oacc_pool,
                    tc.tile_pool(name="psS", bufs=5, space="PSUM") as psS,
                    tc.tile_pool(name="psO", bufs=1, space="PSUM") as psO,
                    tc.tile_pool(name="psR", bufs=1, space="PSUM") as psR,
                    tc.tile_pool(name="psRep", bufs=1, space="PSUM") as psRep,
                ):
                    for h in range(G):
                        qT = q_pool.tile([P, T], F32R, tag="q")
                        nc.sync.dma_start(qT[:], qkd[h])
                        kT = k_pool.tile([P, T], F32R, tag="k")
                        nc.sync.dma_start(kT[:], qkd[G + h])
                        oT = oacc_pool.tile([P, T], F32R, tag="oacc")
                        for Q in range(NQ):
                            jmax = JPQ * (Q + 1) - 1  # inclusive
                            racc = racc_pool.tile([P, QW], F32R, tag="racc")
                            pso = psO.tile([P, QW], F32, tag="pso")
                            # PV(J) consumes exp(S(J)) from ACT; emit it LOOK
                            # S-matmuls later so PE never stalls on ACT.
                            LOOK = 3
                            pend = []
                            for J in range(jmax + 1 + LOOK):
                                if J <= jmax:
                                    k_d = J - JPQ * Q  # diag idx if >= 0
                                    co = max(k_d, 0) * P
                                    pss = psS.tile([P, QW], F32, tag="pss")
                                    nc.tensor.matmul(
                                        pss[:, co:],
                                        kT[:, J * P : (J + 1) * P],
                                        qT[:, Q * QW + co : (Q + 1) * QW],
                                        start=True,
                                        stop=True,
                                        skip_group_check=True,
                                    )
                                    pT = pt_pool.tile([P, QW], F32R, tag="pt")
                                    nc.scalar.activation(
                                        pT[:, co:], pss[:, co:],
                                        mybir.ActivationFunctionType.Exp,
                                        scale=scale,
                                    )
                                    if k_d >= 0:
                                        nc.vector.tensor_mul(
                                            pT[:, co : co + P],
                                            pT[:, co : co + P],
                                            mask_s[:],
                                        )
                                    if J == 0:
                                        nc.any.tensor_copy(racc[:], pT[:])
                                    else:
                                        nc.vector.tensor_add(
                                            racc[:, co:], racc[:, co:],
                                            pT[:, co:],
                                        )
                                    pend.append((J, co, pT))
                                if J >= LOOK:
                                    Jp, cop, pTp = pend.pop(0)
                                    nc.tensor.matmul(
                                        pso[:, cop:],
                                        v_all[:, Jp, h * P : (h + 1) * P],
                                        pTp[:, cop:],
                                        start=(Jp == 0),
                                        stop=(Jp == jmax),
                                        skip_group_check=True,
                                    )
                            psr = psR.tile([1, QW], F32, tag="psr")
                            nc.tensor.matmul(
                                psr[:], one_col[:], racc[:],
                                start=True, stop=True, skip_group_check=True,
                            )
                            rsb = rsb_pool.tile([1, QW], F32, tag="rsb")
                            nc.any.tensor_copy(rsb[:], psr[:])
                            # fold [1,QW] -> [128,QW/128] via DRAM so the
                            # (free-size-bound) reciprocal runs on QW/128
                            # elems per lane instead of QW on one lane
                            FD = QW // P
                            rqda = dramq.tile([P, FD], F32, tag="rqda",
                                              name="rqda")
                            nc.gpsimd.dma_start(
                                rqda.rearrange("a b -> (a b)")[None, :],
                                rsb[:],
                            )
                            rq = rq_pool.tile([P, FD], F32, tag="rq")
                            nc.gpsimd.dma_start(rq[:], rqda[:])
                            rqr = rq_pool.tile([P, FD], F32R, tag="rqr")
                            with nc.allow_low_precision(reason="f32r is 4B"):
                                nc.vector.reciprocal(rqr[:], rq[:])
                            rqdb = dramq.tile([P, FD], F32R, tag="rqdb",
                                              name="rqdb")
                            nc.gpsimd.dma_start(rqdb[:], rqr[:])
                            rinv1 = rsb_pool.tile([1, QW], F32R, tag="rinv1")
                            nc.gpsimd.dma_start(
                                rinv1[:],
                                rqdb.rearrange("a b -> (a b)")[None, :],
                            )
                            psrep = psRep.tile([P, QW], F32, tag="psrep")
                            nc.tensor.matmul(
                                psrep[:], one_row[:], rinv1[:],
                                start=True, stop=True, skip_group_check=True,
                            )
                            rinv = rinv_pool.tile([P, QW], F32, tag="rinv")
                            nc.any.tensor_copy(rinv[:], psrep[:])
                            nc.vector.tensor_mul(
                                oT[:, Q * QW : (Q + 1) * QW], pso[:], rinv[:]
                            )
                        nc.scalar.dma_start(od[h], oT[:])

            # ------------- Phase C: output projection -------------
            with (
                tc.tile_pool(name="wp", bufs=1) as wp_pool,
                tc.tile_pool(name="otc", bufs=2) as otc_pool,
                tc.tile_pool(name="csb", bufs=4) as csb_pool,
                tc.tile_pool(name="psC", bufs=4, space="PSUM") as psC,
            ):
                wp_s = wp_pool.tile([P, KO, G, P], F32R, tag="wp")
                for m in range(KO):
                    nc.sync.dma_start(
                        wp_s[:, m], wp[m]
                    )
                for t in range(NQ):
                    oTt = otc_pool.tile([P, G, QW], F32R, tag="otc")
                    nc.sync.dma_start(
                        oTt[:],
                        od.rearrange("h p t -> p h t")[
                            :, :, t * QW : (t + 1) * QW
                        ],
                    )
                    for m in range(KO):
                        psc = psC.tile([P, QW], F32, tag="psc")
                        for h in range(G):
                            nc.tensor.matmul(
                                psc[:],
                                wp_s[:, m, h, :],
                                oTt[:, h, :],
                                start=(h == 0),
                                stop=(h == G - 1),
                            )
                        csb = csb_pool.tile([P, QW], F32, tag="csb")
                        nc.any.tensor_copy(csb[:], psc[:])
                        nc.sync.dma_start(
                            outT[m * P : (m + 1) * P, t * QW : (t + 1) * QW],
                            csb[:],
                        )

    _split_multi_waits(nc)
    return nc


# --------------------------------------------------------------------------
def _prep_core_inputs(xb, w_attn, w_proj, rope_cos, rope_sin, g, G=8, n_half=2):
    """Host-side shard prep for one core: batch slice xb (T, C), group g."""
    T, C = xb.shape
    KO = C // P
    TH = T // n_half
    VN = min(512, G * P)
    NV = (G * P) // VN
    gc = g * G * P  # column offset of this group within one qkv section

    # x^T arranged [half, ki, ko, t]
    xtT = np.ascontiguousarray(xb.T)  # (C, T)
    xt = np.ascontiguousarray(
        xtT.reshape(KO, P, n_half, TH).transpose(2, 1, 0, 3)
    )

    # q,k columns for this group, RoPE pair-permuted (2i,2i+1) -> (i,64+i)
    perm = np.empty(P, dtype=np.int64)
    perm[:64] = np.arange(0, P, 2)
    perm[64:] = np.arange(1, P, 2)
    wq = w_attn[:, gc : gc + G * P].reshape(C, G, P)[:, :, perm]
    wk = w_attn[:, C + gc : C + gc + G * P].reshape(C, G, P)[:, :, perm]
    wqk_cols = np.concatenate(
        [wq.reshape(C, G * P), wk.reshape(C, G * P)], axis=1
    )  # (C, 2*G*128)
    wqk = np.ascontiguousarray(
        wqk_cols.reshape(KO, P, 2 * G, P).transpose(2, 1, 0, 3)
    )

    wv_cols = w_attn[:, 2 * C + gc : 2 * C + gc + G * P]  # (C, G*128)
    wv = np.ascontiguousarray(
        wv_cols.reshape(KO, P, NV, VN).transpose(2, 1, 0, 3)
    )

    wp_rows = w_proj[gc : gc + G * P, :]  # (G*128, C)
    wp = np.ascontiguousarray(
        wp_rows.reshape(G, P, KO, P).transpose(2, 1, 0, 3)
    )

    cT = rope_cos[:T].T  # (64, T)
    sT = rope_sin[:T].T
    cospT = np.ascontiguousarray(np.concatenate([cT, cT], axis=0))  # (128, T)
    sinpT = np.ascontiguousarray(np.concatenate([-sT, sT], axis=0))
    mask = np.triu(np.ones((P, P), dtype=np.float32))

    return {
        "xt": xt.astype(np.float32),
        "wqk": wqk.astype(np.float32),
        "wv": wv.astype(np.float32),
        "wp": wp.astype(np.float32),
        "cosp": cospT.astype(np.float32),
        "sinp": sinpT.astype(np.float32),
        "maskt": mask,
        "ones_s": np.ones((P, 1), dtype=np.float32),
        "ones1": np.ones((1, P), dtype=np.float32),
    }


_NC_CACHE = {}
TRACE = False
LAST_RESULTS = None


def kernel(x, w_attn, w_proj, rope_cos, rope_sin):
    from concourse.bass_utils import run_bass_kernel_spmd

    x = np.asarray(x, dtype=np.float32)
    w_attn = np.asarray(w_attn, dtype=np.float32)
    w_proj = np.asarray(w_proj, dtype=np.float32)
    rope_cos = np.asarray(rope_cos, dtype=np.float32)
    rope_sin = np.asarray(rope_sin, dtype=np.float32)

    B, T, C = x.shape
    G = 8  # heads per group (16 heads / 2 groups)

    key = (T, C, G)
    if key not in _NC_CACHE:
        _NC_CACHE[key] = build_attention_core(T=T, C=C, G=G, n_half=2)
    nc = _NC_CACHE[key]

    in_maps = []
    for core in range(8):
        b, g = core // 2, core % 2
        in_maps.append(
            _prep_core_inputs(x[b], w_attn, w_proj, rope_cos, rope_sin, g, G=G)
        )

    res = run_bass_kernel_spmd(nc, in_maps, list(range(8)), trace=TRACE)
    global LAST_RESULTS
    LAST_RESULTS = res

    y = np.empty((B, T, C), dtype=np.float32)
    for b in range(B):
        acc = res.results[2 * b]["outT"] + res.results[2 * b + 1]["outT"]
        y[b] = acc.T
    return y



# revision 14
# speedup vs baseline: 1.3365x; 1.3365x over previous
"""Causal self-attention with RoPE on 8 Trainium2 NeuronCores.

Problem (hardcoded): x (4, 2048, 2048) f32, w_attn (2048, 6144),
w_proj (2048, 2048), rope_cos/rope_sin (2048, 64), 16 heads, hd=128.

Sharding: 8 cores = 4 batches x 2 head-groups (8 heads each).  Each core
computes the qkv projection for its heads, RoPE, causal attention, and a
partial output projection; the host sums the two partials per batch.

v2 design (vs the fp32r v1 baseline):
  - QKV projection in fp8(e4m3) split precision: x ~ x_hi + x_lo,
    w ~ w_hi + w_lo (host-prepped, w scaled by 32 to stay in fp8 normal
    range), computed as x_hi@w_hi + x_lo@w_hi + x_hi@w_lo with DoubleRow
    matmuls (2 contraction planes per instr at 0.5 cyc/row) -> 0.75x the
    fp32r PE time with ~0.4% error.
  - Everything else fp16: same 1.0 cyc/row PE rate as fp32r but half the
    DMA/SBUF traffic and 2x DVE element rate; PSUM accumulation stays f32.
  - Softmax normalization deferred per head and software-pipelined into
    the next head's S/PV stream, so the PE queue never stalls on the
    rowsum -> reciprocal -> broadcast chain (v1 lost ~10us x 32 to this).
  - S scores computed full-width (diagonal blocks included); causality is
    one fp16 masked multiply per diagonal tile from a sliced static mask.
  - exp() runs one ACT per J-tile pair over a 2-bank PSUM region.
  - racc (softmax denominator) accumulation split across DVE and Pool
    engines; row sums via [1,512] fp16 matmuls; reciprocal on a DMA-folded
    [128,16] tile; broadcast back via a [1,P]-stationary matmul.
  - v_all and o_all stay SBUF-resident; phase C consumes o_all directly.
"""

import sys
from types import SimpleNamespace

sys.path.insert(0, "/opt/trn_rl_repo")

import numpy as np
import ml_dtypes

import concourse.bass as bass
import concourse.mybir as mybir
import concourse.tile as tile

F32 = mybir.dt.float32
F16 = mybir.dt.float16
F8 = mybir.dt.float8e4
DR = mybir.MatmulPerfMode.DoubleRow
P = 128
WS = 32.0  # host-side scale on w_attn before fp8 split


# --------------------------------------------------------------------------
# This container's walrus build rejects any instruction carrying more than
# one sem wait.  Split extras onto NoOps inserted before the instruction on
# the same engine (per-engine program order makes the waits complete first).
def _split_multi_waits(nc):
    n = 0
    for fn in nc.m.functions:
        for bb in fn.blocks:
            out = []
            changed = False
            for inst in bb.instructions:
                si = inst.sync_info
                waits = list(si.on_wait or []) if si is not None else []
                if len(waits) > 1:
                    changed = True
                    n += 1
                    for w in waits[:-1]:
                        nop = mybir.InstNoOp(
                            name=nc.get_next_instruction_name(),
                            engine=inst.engine,
                            ins=[],
                            outs=[],
                            sync_info=mybir.SyncInfo(on_wait=[w], on_update=[]),
                        )
                        try:
                            nc.register_instruction(nop, overwrite=True)
                        except Exception:
                            pass
                        out.append(nop)
                    inst.sync_info = mybir.SyncInfo(
                        on_wait=[waits[-1]], on_update=list(si.on_update or [])
                    )
                out.append(inst)
            if changed:
                bb.instructions = out
    return n


def _qkv_drs(nc, KO, pss_region, lhs_of, rhs_of):
    """Emit the 24 3-term DoubleRow matmuls for one [*,512] psum region.
    lhs_of/rhs_of map (kind, kc) -> AP; kind 0 = (hi,lo)x(hi,hi) pair at kc,
    kind 1 = (hi,hi)x(lo,lo) across (kc, kc+1)."""
    n = 0
    total = KO + KO // 2
    for kc in range(KO):
        nc.tensor.matmul(
            pss_region, lhs_of(0, kc), rhs_of(0, kc),
            start=(n == 0), stop=(n == total - 1),
            perf_mode=DR, skip_group_check=True,
        )
        n += 1
    for kc in range(0, KO, 2):
        nc.tensor.matmul(
            pss_region, lhs_of(1, kc), rhs_of(1, kc),
            start=(n == 0), stop=(n == total - 1),
            perf_mode=DR, skip_group_check=True,
        )
        n += 1


def _phase_a(g):
    """QKV projection + RoPE; writes qkd (DRAM) and v_all (SBUF)."""
    nc = g.nc
    KO, TH, NTB, NV = g.KO, g.TH, g.NTB, g.NV
    wv_s = g.wv_pool.tile([P, KO, 3, g.G * P], F8, tag="wv")
    nc.sync.dma_start(wv_s[:], g.wv8[:])

    for H in range(g.n_half):
        t0 = H * TH
        xq = g.xt_pool.tile([P, KO, 2, TH], F8, tag="xt")
        nc.sync.dma_start(xq[:], g.xt8[H])

        for m in range(2 * g.G):
            w_s = g.wqk_pool.tile([P, KO, 3, P], F8, tag="wqk")
            nc.sync.dma_start(w_s[:], g.wqk8[m])
            pss = g.psA.tile([P, TH], F32, tag="pssA")
            for cc in range(TH // 512):
                csl = slice(cc * 512, (cc + 1) * 512)
                _qkv_drs(
                    nc, KO, pss[:, csl],
                    lambda kind, kc: (
                        w_s[:, kc, 0:2, :] if kind == 0
                        else w_s[:, kc : kc + 2, 2, :]
                    ),
                    lambda kind, kc, csl=csl: (
                        xq[:, kc, :, csl] if kind == 0
                        else xq[:, kc : kc + 2, 0, csl]
                    ),
                )
            # RoPE: rop = raw*[c;c]/WS + swap(raw)*[-s;+s]/WS
            raw = g.qkraw_pool.tile([P, TH], F16, tag="qkraw")
            nc.scalar.activation(
                raw[:], pss[:], mybir.ActivationFunctionType.Copy
            )
            rop = g.roped_pool.tile([P, TH], F16, tag="roped")
            nc.vector.tensor_mul(rop[:], pss[:], g.cos_s[:, t0 : t0 + TH])
            sw = g.rtmp_pool.tile([P, TH], F16, tag="rtmp")
            nc.sync.dma_start(sw[0:64, :], raw[64:128, :])
            nc.sync.dma_start(sw[64:128, :], raw[0:64, :])
            nc.vector.tensor_mul(sw[:], sw[:], g.sin_s[:, t0 : t0 + TH])
            nc.vector.tensor_add(rop[:], rop[:], sw[:])
            nc.scalar.dma_start(g.qkd[m, :, t0 : t0 + TH], rop[:])

        # v (SBUF-resident, natural [t, d] layout, carries the x32 scale)
        for n2 in range(NV):
            vsl = slice(n2 * 512, (n2 + 1) * 512)
            for tb in range(NTB):
                tsl = slice(tb * P, (tb + 1) * P)
                psv = g.psV.tile([P, 512], F32, tag="pv")
                _qkv_drs(
                    nc, KO, psv[:],
                    lambda kind, kc, tsl=tsl: (
                        xq[:, kc, :, tsl] if kind == 0
                        else xq[:, kc : kc + 2, 0, tsl]
                    ),
                    lambda kind, kc, vsl=vsl: (
                        wv_s[:, kc, 0:2, vsl] if kind == 0
                        else wv_s[:, kc : kc + 2, 2, vsl]
                    ),
                )
                # un-scale the x32 here: leaving it on v can overflow fp16
                # in the unnormalized P@v when the softmax row is spiky
                nc.scalar.activation(
                    g.v_all[:, H * NTB + tb, vsl], psv[:],
                    mybir.ActivationFunctionType.Copy, scale=1.0 / WS,
                )


def _load_qk(g, h):
    nc = g.nc
    qT = g.q_pool.tile([P, g.T], F16, tag="q")
    nc.sync.dma_start(qT[:], g.qkd[h])
    kT = g.k_pool.tile([P, g.T], F16, tag="k")
    nc.sync.dma_start(kT[:], g.qkd[g.G + h])
    return qT, kT


def _make_norm_stages(g, h, racc_d, racc_p):
    """Deferred normalization of head h; four stages injected into head
    h+1's instruction stream."""
    nc = g.nc
    NQ, QW = g.NQ, g.QW
    rs = g.rs_pool.tile([1, g.T], F32, tag="rs")
    rinv1 = g.rinv1_pool.tile([1, g.T], F16, tag="rinv1")

    def s_psr():
        nc.vector.tensor_add(racc_d[:], racc_d[:], racc_p[:])
        for c in range(NQ):
            csl = slice(c * QW, (c + 1) * QW)
            psn = g.psN.tile([P, QW], F32, tag="psn")
            nc.tensor.matmul(
                psn[0:1, :], g.onec_s[:], racc_d[:, csl],
                start=True, stop=True, skip_group_check=True,
            )
            nc.vector.tensor_copy(rs[:, csl], psn[0:1, :])

    def s_fold():
        rqd = g.dramq.tile([P, 16], F32, tag="rqd", name=f"rqd{h}")
        nc.gpsimd.dma_start(rqd.rearrange("a b -> (a b)")[None, :], rs[:])
        rq = g.rq_pool.tile([P, 16], F32, tag="rq")
        nc.gpsimd.dma_start(rq[:], rqd[:])
        rr = g.rq_pool.tile([P, 16], F16, tag="rr")
        nc.vector.reciprocal(rr[:], rq[:])
        rrd = g.dramq.tile([P, 16], F16, tag="rrd", name=f"rrd{h}")
        nc.gpsimd.dma_start(rrd[:], rr[:])
        nc.gpsimd.dma_start(
            rinv1[:], rrd.rearrange("a b -> (a b)")[None, :]
        )

    def s_psrep_mul():
        # broadcast 1/rowsum via matmul, multiply o from PSUM right away so
        # the psN pool (bufs=2) never holds more than 2 live tiles
        for c in range(NQ):
            csl = slice(c * QW, (c + 1) * QW)
            psb = g.psN.tile([P, QW], F32, tag="psn")
            nc.tensor.matmul(
                psb[:], g.oner_s[:], rinv1[0:1, csl],
                start=True, stop=True, skip_group_check=True,
            )
            nc.vector.tensor_mul(
                g.o_all[:, h, csl], g.o_all[:, h, csl], psb[:]
            )

    return [s_psr, s_fold, s_psrep_mul]


def _head_stream(g, h, qT, kT, pending_norm, prefetch):
    """One head's S/exp/mask/racc/PV stream with norm-stage injection.
    Returns this head's norm stages."""
    nc = g.nc
    NQ, QW, JPQ = g.NQ, g.QW, g.JPQ
    hcol = slice(h * P, (h + 1) * P)
    racc_d = g.raccd_pool.tile([P, g.T], F16, tag="rd")
    racc_p = g.raccp_pool.tile([P, g.T], F16, tag="rp")
    started_d = [False] * NQ
    started_p = [False] * NQ
    pend = []

    def pop_pv():
        (Q_, J0_, J1_, pT_, pso_, last_) = pend.pop(0)
        jmax_ = JPQ * (Q_ + 1) - 1
        nc.tensor.matmul(
            pso_[:], g.v_all[:, J0_, hcol], pT_[:, 0:512],
            start=(J0_ == 0), stop=False, skip_group_check=True,
        )
        nc.tensor.matmul(
            pso_[:], g.v_all[:, J1_, hcol], pT_[:, 512:1024],
            start=False, stop=(J1_ == jmax_), skip_group_check=True,
        )
        if last_:
            nc.scalar.activation(
                g.o_all[:, h, Q_ * QW : (Q_ + 1) * QW], pso_[:],
                mybir.ActivationFunctionType.Copy,
            )

    INJECT = {2: 0, 5: 1, 9: 2}
    slot = 0
    for Q in range(NQ):
        qsl = slice(Q * QW, (Q + 1) * QW)
        pso = g.psO.tile([P, QW], F32, tag="pso")
        npairs = JPQ * (Q + 1) // 2
        for p_i in range(npairs):
            if pending_norm is not None and slot in INJECT:
                pending_norm[INJECT[slot]]()
            if slot == 6:
                prefetch()
            J0, J1 = 2 * p_i, 2 * p_i + 1
            pss = g.psS.tile([P, 1024], F32, tag="pss")
            nc.tensor.matmul(
                pss[:, 0:512], kT[:, J0 * P : (J0 + 1) * P], qT[:, qsl],
                start=True, stop=True, skip_group_check=True,
            )
            nc.tensor.matmul(
                pss[:, 512:1024], kT[:, J1 * P : (J1 + 1) * P], qT[:, qsl],
                start=True, stop=True, skip_group_check=True,
            )
            pT = g.pt_pool.tile([P, 1024], F16, tag="pt")
            nc.scalar.activation(
                pT[:], pss[:], mybir.ActivationFunctionType.Exp,
                scale=g.scale,
            )
            for s, J in ((0, J0), (1, J1)):
                psl = slice(s * 512, (s + 1) * 512)
                if J >= JPQ * Q:  # diagonal tile: mask [0, co+128)
                    co = (J - JPQ * Q) * P
                    nc.vector.tensor_mul(
                        pT[:, s * 512 : s * 512 + co + P],
                        pT[:, s * 512 : s * 512 + co + P],
                        g.mz_s[:, 384 - co :],
                    )
                tj = 2 * p_i + s
                if tj % 3 < 2:
                    eng, st, racc = nc.vector, started_d, racc_d
                else:
                    eng, st, racc = nc.gpsimd, started_p, racc_p
                if not st[Q]:
                    eng.tensor_copy(racc[:, qsl], pT[:, psl])
                    st[Q] = True
                else:
                    eng.tensor_add(racc[:, qsl], racc[:, qsl], pT[:, psl])
            pend.append((Q, J0, J1, pT, pso, p_i == npairs - 1))
            if len(pend) > g.LOOK:
                pop_pv()
            slot += 1
    while pend:
        pop_pv()
    return _make_norm_stages(g, h, racc_d, racc_p)


def _phase_b(g):
    nc = g.nc
    qkt = _load_qk(g, 0)
    pending_norm = None
    nxt = [None]
    for h in range(g.G):
        def prefetch(h=h):
            if h + 1 < g.G:
                nxt[0] = _load_qk(g, h + 1)
        pending_norm = _head_stream(
            g, h, qkt[0], qkt[1], pending_norm, prefetch
        )
        if nxt[0] is not None:
            qkt = nxt[0]
            nxt[0] = None
    for stage in pending_norm:  # last head, unpipelined tail
        stage()


def _phase_c(g):
    nc = g.nc
    for t in range(g.NQ):
        tsl = slice(t * g.QW, (t + 1) * g.QW)
        for m in range(g.KO):
            psc = g.psC.tile([P, g.QW], F32, tag="psc")
            for hh in range(g.G):
                nc.tensor.matmul(
                    psc[:], g.wp_s[:, m, hh, :], g.o_all[:, hh, tsl],
                    start=(hh == 0), stop=(hh == g.G - 1),
                )
            csb = g.csb_pool.tile([P, g.QW], F16, tag="csb")
            nc.scalar.activation(
                csb[:], psc[:], mybir.ActivationFunctionType.Copy
            )
            nc.gpsimd.dma_start(g.outT[m * P : (m + 1) * P, tsl], csb[:])


def build_attention_core(T=2048, C=2048, G=8, n_half=2):
    g = SimpleNamespace()
    g.T, g.C, g.G, g.n_half = T, C, G, n_half
    g.KO = C // P
    g.TH = T // n_half
    g.NTB = g.TH // P
    g.NV = (G * P) // 512
    g.NQ = T // 512
    g.QW = 512
    g.JPQ = g.QW // P
    g.NJ = T // P
    g.LOOK = 2
    g.scale = 1.0 / np.sqrt(128.0)

    nc = bass.Bass()
    g.nc = nc
    g.xt8 = nc.dram_tensor("xt8", [n_half, P, g.KO, 2, g.TH], F8,
                           kind="ExternalInput")
    g.wqk8 = nc.dram_tensor("wqk8", [2 * G, P, g.KO, 3, P], F8,
                            kind="ExternalInput")
    g.wv8 = nc.dram_tensor("wv8", [P, g.KO, 3, G * P], F8,
                           kind="ExternalInput")
    g.wp16 = nc.dram_tensor("wp16", [g.KO, P, G, P], F16,
                            kind="ExternalInput")
    g.cosp = nc.dram_tensor("cosp", [P, T], F16, kind="ExternalInput")
    g.sinp = nc.dram_tensor("sinp", [P, T], F16, kind="ExternalInput")
    g.maskz = nc.dram_tensor("maskz", [P, 512], F16, kind="ExternalInput")
    g.ones_c = nc.dram_tensor("ones_c", [P, 1], F16, kind="ExternalInput")
    g.ones_r = nc.dram_tensor("ones_r", [1, P], F16, kind="ExternalInput")
    g.outT = nc.dram_tensor("outT", [C, T], F16, kind="ExternalOutput")

    with tile.TileContext(nc) as tc, nc.allow_low_precision(
        reason="fp16 kernel"
    ):
        with (
            tc.tile_pool(name="dram", bufs=1, space="DRAM") as dram,
            tc.tile_pool(name="const", bufs=1) as cpool,
        ):
            g.qkd = dram.tile([2 * G, P, T], F16)
            g.cos_s = cpool.tile([P, T], F16)
            nc.sync.dma_start(g.cos_s[:], g.cosp[:])
            g.sin_s = cpool.tile([P, T], F16)
            nc.sync.dma_start(g.sin_s[:], g.sinp[:])
            g.mz_s = cpool.tile([P, 512], F16)
            nc.sync.dma_start(g.mz_s[:], g.maskz[:])
            g.onec_s = cpool.tile([P, 1], F16)
            nc.sync.dma_start(g.onec_s[:], g.ones_c[:])
            g.oner_s = cpool.tile([1, P], F16)
            nc.sync.dma_start(g.oner_s[:], g.ones_r[:])

            with tc.tile_pool(name="vall", bufs=1) as va_pool:
                g.v_all = va_pool.tile([P, g.NJ, G * P], F16, tag="vall")

                with (
                    tc.tile_pool(name="xt", bufs=2) as xt_pool,
                    tc.tile_pool(name="wqk", bufs=2) as wqk_pool,
                    tc.tile_pool(name="wv", bufs=1) as wv_pool,
                    tc.tile_pool(name="qkraw", bufs=2) as qkraw_pool,
                    tc.tile_pool(name="roped", bufs=2) as roped_pool,
                    tc.tile_pool(name="ropetmp", bufs=2) as rtmp_pool,
                    tc.tile_pool(name="psA", bufs=2, space="PSUM") as psA,
                    tc.tile_pool(name="psV", bufs=2, space="PSUM") as psV,
                ):
                    g.xt_pool, g.wqk_pool, g.wv_pool = xt_pool, wqk_pool, wv_pool
                    g.qkraw_pool, g.roped_pool, g.rtmp_pool = (
                        qkraw_pool, roped_pool, rtmp_pool
                    )
                    g.psA, g.psV = psA, psV
                    _phase_a(g)

                with (
                    tc.tile_pool(name="oall", bufs=1) as oa_pool,
                    tc.tile_pool(name="wp", bufs=1) as wp_pool,
                ):
                    g.o_all = oa_pool.tile([P, G, T], F16, tag="oall")
                    g.wp_s = wp_pool.tile([P, g.KO, G, P], F16, tag="wp")
                    for m in range(g.KO):
                        nc.sync.dma_start(g.wp_s[:, m], g.wp16[m])

                    with (
                        tc.tile_pool(name="qh", bufs=2) as q_pool,
                        tc.tile_pool(name="kh", bufs=2) as k_pool,
                        tc.tile_pool(name="pt", bufs=6) as pt_pool,
                        tc.tile_pool(name="raccd", bufs=2) as raccd_pool,
                        tc.tile_pool(name="raccp", bufs=2) as raccp_pool,
                        tc.tile_pool(name="rsb", bufs=2) as rs_pool,
                        tc.tile_pool(name="rinv1", bufs=2) as rinv1_pool,
                        tc.tile_pool(name="rq", bufs=4) as rq_pool,
                        tc.tile_pool(name="dramq", bufs=4, space="DRAM")
                        as dramq,
                        tc.tile_pool(name="psS", bufs=2, space="PSUM") as psS,
                        tc.tile_pool(name="psO", bufs=2, space="PSUM") as psO,
                        tc.tile_pool(name="psN", bufs=2, space="PSUM") as psN,
                    ):
                        g.q_pool, g.k_pool, g.pt_pool = q_pool, k_pool, pt_pool
                        g.raccd_pool, g.raccp_pool = raccd_pool, raccp_pool
                        g.rs_pool, g.rinv1_pool = rs_pool, rinv1_pool
                        g.rq_pool, g.dramq = rq_pool, dramq
                        g.psS, g.psO, g.psN = psS, psO, psN
                        _phase_b(g)

                    with (
                        tc.tile_pool(name="csb", bufs=4) as csb_pool,
                        tc.tile_pool(name="psC", bufs=4, space="PSUM") as psC,
                    ):
                        g.csb_pool, g.psC = csb_pool, psC
                        _phase_c(g)

    _split_multi_waits(nc)
    return nc


# --------------------------------------------------------------------------
def _prep_core_inputs(xb, w_attn, w_proj, rope_cos, rope_sin, gidx, G=8,
                      n_half=2):
    """Host-side shard prep for one core: batch slice xb (T, C), group gidx."""
    T, C = xb.shape
    KO = C // P
    TH = T // n_half
    gc = gidx * G * P
    E4 = ml_dtypes.float8_e4m3

    def split8(a):
        hi = a.astype(E4)
        lo = (a - hi.astype(np.float32)).astype(E4)
        return hi, lo

    # x^T planes [H, p, kc, (hi,lo), t]
    xt = np.ascontiguousarray(xb.T).reshape(KO, P, n_half, TH)
    xhi, xlo = split8(xt)
    xt8 = np.ascontiguousarray(
        np.stack([xhi, xlo], axis=0).transpose(3, 2, 1, 0, 4)
    )

    # q,k columns, RoPE pair-permuted (2i,2i+1) -> (i,64+i), scaled by WS
    perm = np.empty(P, dtype=np.int64)
    perm[:64] = np.arange(0, P, 2)
    perm[64:] = np.arange(1, P, 2)
    wq = w_attn[:, gc : gc + G * P].reshape(C, G, P)[:, :, perm]
    wk = w_attn[:, C + gc : C + gc + G * P].reshape(C, G, P)[:, :, perm]
    wqk_cols = np.concatenate(
        [wq.reshape(C, G * P), wk.reshape(C, G * P)], axis=1
    ) * WS
    whi, wlo = split8(wqk_cols)
    whi = whi.reshape(KO, P, 2 * G, P)
    wlo = wlo.reshape(KO, P, 2 * G, P)
    wqk8 = np.ascontiguousarray(
        np.stack([whi, whi, wlo], axis=0).transpose(3, 2, 1, 0, 4)
    )

    wv_cols = w_attn[:, 2 * C + gc : 2 * C + gc + G * P] * WS
    vhi, vlo = split8(wv_cols)
    vhi = vhi.reshape(KO, P, G * P)
    vlo = vlo.reshape(KO, P, G * P)
    wv8 = np.ascontiguousarray(
        np.stack([vhi, vhi, vlo], axis=0).transpose(2, 1, 0, 3)
    )

    # w_proj rows for this group (v's x32 is un-scaled at the v copy)
    wp_rows = w_proj[gc : gc + G * P, :]
    wp16 = np.ascontiguousarray(
        wp_rows.reshape(G, P, KO, P).transpose(2, 1, 0, 3)
    ).astype(np.float16)

    cT = rope_cos[:T].T / WS
    sT = rope_sin[:T].T / WS
    cosp = np.ascontiguousarray(np.concatenate([cT, cT], axis=0)).astype(
        np.float16
    )
    sinp = np.ascontiguousarray(np.concatenate([-sT, sT], axis=0)).astype(
        np.float16
    )
    maskz = np.concatenate(
        [np.zeros((P, 384), dtype=np.float32), np.triu(np.ones((P, P)))],
        axis=1,
    ).astype(np.float16)

    return {
        "xt8": xt8,
        "wqk8": wqk8,
        "wv8": wv8,
        "wp16": wp16,
        "cosp": cosp,
        "sinp": sinp,
        "maskz": maskz,
        "ones_c": np.ones((P, 1), dtype=np.float16),
        "ones_r": np.ones((1, P), dtype=np.float16),
    }


_NC_CACHE = {}
TRACE = False
LAST_RESULTS = None


def kernel(x, w_attn, w_proj, rope_cos, rope_sin):
    from concourse.bass_utils import run_bass_kernel_spmd

    x = np.asarray(x, dtype=np.float32)
    w_attn = np.asarray(w_attn, dtype=np.float32)
    w_proj = np.asarray(w_proj, dtype=np.float32)
    rope_cos = np.asarray(rope_cos, dtype=np.float32)
    rope_sin = np.asarray(rope_sin, dtype=np.float32)

    B, T, C = x.shape
    G = 8  # heads per group (16 heads / 2 groups)

    key = (T, C, G)
    if key not in _NC_CACHE:
        _NC_CACHE[key] = build_attention_core(T=T, C=C, G=G, n_half=2)
    nc = _NC_CACHE[key]

    in_maps = []
    for core in range(8):
        b, gi = core // 2, core % 2
        in_maps.append(
            _prep_core_inputs(x[b], w_attn, w_proj, rope_cos, rope_sin, gi,
                              G=G)
        )

    res = run_bass_kernel_spmd(nc, in_maps, list(range(8)), trace=TRACE)
    global LAST_RESULTS
    LAST_RESULTS = res

    y = np.empty((B, T, C), dtype=np.float32)
    for b in range(B):
        acc = res.results[2 * b]["outT"].astype(np.float32) + res.results[
            2 * b + 1
        ]["outT"].astype(np.float32)
        y[b] = acc.T
    return y


# revision 23
# speedup vs baseline: 1.6747x; 1.2530x over previous
"""Causal self-attention with RoPE on 8 Trainium2 NeuronCores.

Problem (hardcoded): x (4, 2048, 2048) f32, w_attn (2048, 6144),
w_proj (2048, 2048), rope_cos/rope_sin (2048, 64), 16 heads, hd=128.

Sharding: 8 cores = 4 batches x 2 head-groups (8 heads each).  Each core
computes the qkv projection for its heads, RoPE, causal attention, and a
partial output projection; the host sums the two partials per batch.

v2 design (vs the fp32r v1 baseline):
  - QKV projection in fp8(e4m3) split precision: x ~ x_hi + x_lo,
    w ~ w_hi + w_lo (host-prepped, w scaled by 32 to stay in fp8 normal
    range), computed as x_hi@w_hi + x_lo@w_hi + x_hi@w_lo with DoubleRow
    matmuls (2 contraction planes per instr at 0.5 cyc/row) -> 0.75x the
    fp32r PE time with ~0.4% error.
  - Everything else fp16: same 1.0 cyc/row PE rate as fp32r but half the
    DMA/SBUF traffic and 2x DVE element rate; PSUM accumulation stays f32.
  - Softmax normalization deferred per head and software-pipelined into
    the next head's S/PV stream, so the PE queue never stalls on the
    rowsum -> reciprocal -> broadcast chain (v1 lost ~10us x 32 to this).
  - S scores computed full-width (diagonal blocks included); causality is
    one fp16 masked multiply per diagonal tile from a sliced static mask.
  - exp() runs one ACT per J-tile pair over a 2-bank PSUM region.
  - racc (softmax denominator) accumulation split across DVE and Pool
    engines; row sums via [1,512] fp16 matmuls; reciprocal on a DMA-folded
    [128,16] tile; broadcast back via a [1,P]-stationary matmul.
  - v_all and o_all stay SBUF-resident; phase C consumes o_all directly.
"""

import sys
from types import SimpleNamespace

sys.path.insert(0, "/opt/trn_rl_repo")

import numpy as np
import ml_dtypes

import concourse.bass as bass
import concourse.mybir as mybir
import concourse.tile as tile

F32 = mybir.dt.float32
F16 = mybir.dt.float16
F8 = mybir.dt.float8e4
DR = mybir.MatmulPerfMode.DoubleRow
P = 128
WS = 32.0  # host-side scale on w_attn before fp8 split


# --------------------------------------------------------------------------
# This container's walrus build rejects any instruction carrying more than
# one sem wait.  Split extras onto NoOps inserted before the instruction on
# the same engine (per-engine program order makes the waits complete first).
def _split_multi_waits(nc):
    n = 0
    for fn in nc.m.functions:
        for bb in fn.blocks:
            out = []
            changed = False
            for inst in bb.instructions:
                si = inst.sync_info
                waits = list(si.on_wait or []) if si is not None else []
                if len(waits) > 1:
                    changed = True
                    n += 1
                    for w in waits[:-1]:
                        nop = mybir.InstNoOp(
                            name=nc.get_next_instruction_name(),
                            engine=inst.engine,
                            ins=[],
                            outs=[],
                            sync_info=mybir.SyncInfo(on_wait=[w], on_update=[]),
                        )
                        try:
                            nc.register_instruction(nop, overwrite=True)
                        except Exception:
                            pass
                        out.append(nop)
                    inst.sync_info = mybir.SyncInfo(
                        on_wait=[waits[-1]], on_update=list(si.on_update or [])
                    )
                out.append(inst)
            if changed:
                bb.instructions = out
    return n


def _phase_a(g):
    """QKV projection + RoPE; writes qkd (DRAM) and v_all (SBUF)."""
    nc = g.nc
    KO, TH, NTB, NV = g.KO, g.TH, g.NTB, g.NV
    wv_s = g.wv_pool.tile([P, KO, g.G * P], F16, tag="wv")
    nc.sync.dma_start(wv_s[:], g.wv16[:])

    for H in range(g.n_half):
        t0 = H * TH
        xq = g.xt_pool.tile([P, KO, TH], F16, tag="xt")
        nc.sync.dma_start(xq[:], g.xt16[H])

        for m in range(2 * g.G):
            w_s = g.wqk_pool.tile([P, KO, P], F16, tag="wqk")
            nc.sync.dma_start(w_s[:], g.wqk16[m])
            pss = g.psA.tile([P, TH], F32, tag="pssA")
            for cc in range(TH // 512):
                csl = slice(cc * 512, (cc + 1) * 512)
                for kc in range(KO):
                    nc.tensor.matmul(
                        pss[:, csl], w_s[:, kc, :], xq[:, kc, csl],
                        start=(kc == 0), stop=(kc == KO - 1),
                        skip_group_check=True,
                    )
            # RoPE: rop = raw*[c;c]/WS + swap(raw)*[-s;+s]/WS
            raw = g.qkraw_pool.tile([P, TH], F16, tag="qkraw")
            nc.scalar.activation(
                raw[:], pss[:], mybir.ActivationFunctionType.Copy
            )
            rop = g.roped_pool.tile([P, TH], F16, tag="roped")
            nc.vector.tensor_mul(rop[:], pss[:], g.cos_s[:, t0 : t0 + TH])
            sw = g.rtmp_pool.tile([P, TH], F16, tag="rtmp")
            nc.sync.dma_start(sw[0:64, :], raw[64:128, :])
            nc.sync.dma_start(sw[64:128, :], raw[0:64, :])
            nc.vector.tensor_mul(sw[:], sw[:], g.sin_s[:, t0 : t0 + TH])
            nc.vector.tensor_add(rop[:], rop[:], sw[:])
            nc.scalar.dma_start(g.qkd[m, :, t0 : t0 + TH], rop[:])

        # v (SBUF-resident, natural [t, d] layout)
        for n2 in range(NV):
            vsl = slice(n2 * 512, (n2 + 1) * 512)
            for tb in range(NTB):
                tsl = slice(tb * P, (tb + 1) * P)
                psv = g.psV.tile([P, 512], F32, tag="pv")
                for kc in range(KO):
                    nc.tensor.matmul(
                        psv[:], xq[:, kc, tsl], wv_s[:, kc, vsl],
                        start=(kc == 0), stop=(kc == KO - 1),
                        skip_group_check=True,
                    )
                nc.scalar.activation(
                    g.v_all[:, H * NTB + tb, vsl], psv[:],
                    mybir.ActivationFunctionType.Copy,
                )


def _load_qk(g, h):
    nc = g.nc
    qT = g.q_pool.tile([P, g.T], F16, tag="q")
    nc.sync.dma_start(qT[:], g.qkd[h])
    kT = g.k_pool.tile([P, g.T], F16, tag="k")
    nc.sync.dma_start(kT[:], g.qkd[g.G + h])
    return qT, kT


def _make_norm_stages(g, h, racc_d, racc_p):
    """Deferred normalization of head h; four stages injected into head
    h+1's instruction stream."""
    nc = g.nc
    NQ, QW = g.NQ, g.QW
    rs = g.rs_pool.tile([1, g.T], F32, tag="rs")
    rinv1 = g.rinv1_pool.tile([1, g.T], F16, tag="rinv1")

    def s_psr():
        nc.vector.tensor_add(racc_d[:], racc_d[:], racc_p[:])
        for c in range(NQ):
            csl = slice(c * QW, (c + 1) * QW)
            psn = g.psN.tile([P, QW], F32, tag="psn")
            nc.tensor.matmul(
                psn[0:1, :], g.onec_s[:], racc_d[:, csl],
                start=True, stop=True, skip_group_check=True,
            )
            nc.vector.tensor_copy(rs[:, csl], psn[0:1, :])

    def s_fold():
        rqd = g.dramq.tile([P, 16], F32, tag="rqd", name=f"rqd{h}")
        nc.gpsimd.dma_start(rqd.rearrange("a b -> (a b)")[None, :], rs[:])
        rq = g.rq_pool.tile([P, 16], F32, tag="rq")
        nc.gpsimd.dma_start(rq[:], rqd[:])
        rr = g.rq_pool.tile([P, 16], F16, tag="rr")
        nc.vector.reciprocal(rr[:], rq[:])
        rrd = g.dramq.tile([P, 16], F16, tag="rrd", name=f"rrd{h}")
        nc.gpsimd.dma_start(rrd[:], rr[:])
        nc.gpsimd.dma_start(
            rinv1[:], rrd.rearrange("a b -> (a b)")[None, :]
        )

    def s_psrep_mul():
        # broadcast 1/rowsum via matmul, multiply o from PSUM right away so
        # the psN pool (bufs=2) never holds more than 2 live tiles
        for c in range(NQ):
            csl = slice(c * QW, (c + 1) * QW)
            psb = g.psN.tile([P, QW], F32, tag="psn")
            nc.tensor.matmul(
                psb[:], g.oner_s[:], rinv1[0:1, csl],
                start=True, stop=True, skip_group_check=True,
            )
            nc.vector.tensor_mul(
                g.o_all[:, h, csl], g.o_all[:, h, csl], psb[:]
            )

    return [s_psr, s_fold, s_psrep_mul]


def _head_stream(g, h, qT, kT, pending_norm, prefetch):
    """One head's S/exp/mask/racc/PV stream with norm-stage injection.
    Returns this head's norm stages."""
    nc = g.nc
    NQ, QW, JPQ = g.NQ, g.QW, g.JPQ
    hcol = slice(h * P, (h + 1) * P)
    racc_d = g.raccd_pool.tile([P, g.T], F16, tag="rd")
    racc_p = g.raccp_pool.tile([P, g.T], F16, tag="rp")
    started_d = [False] * NQ
    started_p = [False] * NQ
    pend = []

    def pop_pv():
        (Q_, J0_, J1_, pT_, pso_, last_) = pend.pop(0)
        jmax_ = JPQ * (Q_ + 1) - 1
        nc.tensor.matmul(
            pso_[:], g.v_all[:, J0_, hcol], pT_[:, 0:512],
            start=(J0_ == 0), stop=False, skip_group_check=True,
        )
        nc.tensor.matmul(
            pso_[:], g.v_all[:, J1_, hcol], pT_[:, 512:1024],
            start=False, stop=(J1_ == jmax_), skip_group_check=True,
        )
        if last_:
            nc.scalar.activation(
                g.o_all[:, h, Q_ * QW : (Q_ + 1) * QW], pso_[:],
                mybir.ActivationFunctionType.Copy,
            )

    INJECT = {2: 0, 5: 1, 9: 2}
    slot = 0
    for Q in range(NQ):
        qsl = slice(Q * QW, (Q + 1) * QW)
        pso = g.psO.tile([P, QW], F32, tag="pso")
        npairs = JPQ * (Q + 1) // 2
        for p_i in range(npairs):
            if pending_norm is not None and slot in INJECT:
                pending_norm[INJECT[slot]]()
            if slot == 6:
                prefetch()
            J0, J1 = 2 * p_i, 2 * p_i + 1
            pss = g.psS.tile([P, 1024], F32, tag="pss")
            nc.tensor.matmul(
                pss[:, 0:512], kT[:, J0 * P : (J0 + 1) * P], qT[:, qsl],
                start=True, stop=True, skip_group_check=True,
            )
            nc.tensor.matmul(
                pss[:, 512:1024], kT[:, J1 * P : (J1 + 1) * P], qT[:, qsl],
                start=True, stop=True, skip_group_check=True,
            )
            pT = g.pt_pool.tile([P, 1024], F16, tag="pt")
            nc.scalar.activation(
                pT[:], pss[:], mybir.ActivationFunctionType.Exp,
                scale=g.scale,
            )
            for s, J in ((0, J0), (1, J1)):
                psl = slice(s * 512, (s + 1) * 512)
                if J >= JPQ * Q:  # diagonal tile: mask [0, co+128)
                    co = (J - JPQ * Q) * P
                    nc.vector.tensor_mul(
                        pT[:, s * 512 : s * 512 + co + P],
                        pT[:, s * 512 : s * 512 + co + P],
                        g.mz_s[:, 384 - co :],
                    )
                tj = 2 * p_i + s
                if tj % 3 < 2:
                    eng, st, racc = nc.vector, started_d, racc_d
                else:
                    eng, st, racc = nc.gpsimd, started_p, racc_p
                if not st[Q]:
                    eng.tensor_copy(racc[:, qsl], pT[:, psl])
                    st[Q] = True
                else:
                    eng.tensor_add(racc[:, qsl], racc[:, qsl], pT[:, psl])
            pend.append((Q, J0, J1, pT, pso, p_i == npairs - 1))
            if len(pend) > g.LOOK:
                pop_pv()
            slot += 1
    while pend:
        pop_pv()
    return _make_norm_stages(g, h, racc_d, racc_p)


def _phase_b(g):
    nc = g.nc
    qkt = _load_qk(g, 0)
    # wp for phase C: off the sync queue so it doesn't delay the q/k loads
    for m in range(g.KO):
        nc.gpsimd.dma_start(g.wp_s[:, m], g.wp16[m])
    pending_norm = None
    nxt = [None]
    for h in range(g.G):
        def prefetch(h=h):
            if h + 1 < g.G:
                nxt[0] = _load_qk(g, h + 1)
        pending_norm = _head_stream(
            g, h, qkt[0], qkt[1], pending_norm, prefetch
        )
        if nxt[0] is not None:
            qkt = nxt[0]
            nxt[0] = None
    for stage in pending_norm:  # last head, unpipelined tail
        stage()


def _phase_c(g):
    nc = g.nc
    for t in range(g.NQ):
        tsl = slice(t * g.QW, (t + 1) * g.QW)
        for m in range(g.KO):
            psc = g.psC.tile([P, g.QW], F32, tag="psc")
            for hh in range(g.G):
                nc.tensor.matmul(
                    psc[:], g.wp_s[:, m, hh, :], g.o_all[:, hh, tsl],
                    start=(hh == 0), stop=(hh == g.G - 1),
                )
            csb = g.csb_pool.tile([P, g.QW], F16, tag="csb")
            nc.scalar.activation(
                csb[:], psc[:], mybir.ActivationFunctionType.Copy
            )
            nc.gpsimd.dma_start(g.outT[m * P : (m + 1) * P, tsl], csb[:])


def build_attention_core(T=2048, C=2048, G=8, n_half=2):
    g = SimpleNamespace()
    g.T, g.C, g.G, g.n_half = T, C, G, n_half
    g.KO = C // P
    g.TH = T // n_half
    g.NTB = g.TH // P
    g.NV = (G * P) // 512
    g.NQ = T // 512
    g.QW = 512
    g.JPQ = g.QW // P
    g.NJ = T // P
    g.LOOK = 4
    g.scale = 1.0 / np.sqrt(128.0)

    nc = bass.Bass()
    g.nc = nc
    g.xt16 = nc.dram_tensor("xt16", [n_half, P, g.KO, g.TH], F16,
                            kind="ExternalInput")
    g.wqk16 = nc.dram_tensor("wqk16", [2 * G, P, g.KO, P], F16,
                             kind="ExternalInput")
    g.wv16 = nc.dram_tensor("wv16", [P, g.KO, G * P], F16,
                            kind="ExternalInput")
    g.wp16 = nc.dram_tensor("wp16", [g.KO, P, G, P], F16,
                            kind="ExternalInput")
    g.cosp = nc.dram_tensor("cosp", [P, T], F16, kind="ExternalInput")
    g.sinp = nc.dram_tensor("sinp", [P, T], F16, kind="ExternalInput")
    g.maskz = nc.dram_tensor("maskz", [P, 512], F16, kind="ExternalInput")
    g.ones_c = nc.dram_tensor("ones_c", [P, 1], F16, kind="ExternalInput")
    g.ones_r = nc.dram_tensor("ones_r", [1, P], F16, kind="ExternalInput")
    g.outT = nc.dram_tensor("outT", [C, T], F16, kind="ExternalOutput")

    with tile.TileContext(nc) as tc, nc.allow_low_precision(
        reason="fp16 kernel"
    ):
        with (
            tc.tile_pool(name="dram", bufs=1, space="DRAM") as dram,
            tc.tile_pool(name="const", bufs=1) as cpool,
        ):
            g.qkd = dram.tile([2 * G, P, T], F16)
            g.cos_s = cpool.tile([P, T], F16)
            nc.sync.dma_start(g.cos_s[:], g.cosp[:])
            g.sin_s = cpool.tile([P, T], F16)
            nc.sync.dma_start(g.sin_s[:], g.sinp[:])
            g.mz_s = cpool.tile([P, 512], F16)
            nc.sync.dma_start(g.mz_s[:], g.maskz[:])
            g.onec_s = cpool.tile([P, 1], F16)
            nc.sync.dma_start(g.onec_s[:], g.ones_c[:])
            g.oner_s = cpool.tile([1, P], F16)
            nc.sync.dma_start(g.oner_s[:], g.ones_r[:])

            with tc.tile_pool(name="vall", bufs=1) as va_pool:
                g.v_all = va_pool.tile([P, g.NJ, G * P], F16, tag="vall")

                with (
                    tc.tile_pool(name="xt", bufs=2) as xt_pool,
                    tc.tile_pool(name="wqk", bufs=2) as wqk_pool,
                    tc.tile_pool(name="wv", bufs=1) as wv_pool,
                    tc.tile_pool(name="qkraw", bufs=2) as qkraw_pool,
                    tc.tile_pool(name="roped", bufs=2) as roped_pool,
                    tc.tile_pool(name="ropetmp", bufs=2) as rtmp_pool,
                    tc.tile_pool(name="psA", bufs=2, space="PSUM") as psA,
                    tc.tile_pool(name="psV", bufs=2, space="PSUM") as psV,
                ):
                    g.xt_pool, g.wqk_pool, g.wv_pool = xt_pool, wqk_pool, wv_pool
                    g.qkraw_pool, g.roped_pool, g.rtmp_pool = (
                        qkraw_pool, roped_pool, rtmp_pool
                    )
                    g.psA, g.psV = psA, psV
                    _phase_a(g)

                with (
                    tc.tile_pool(name="oall", bufs=1) as oa_pool,
                    tc.tile_pool(name="wp", bufs=1) as wp_pool,
                ):
                    g.o_all = oa_pool.tile([P, G, T], F16, tag="oall")
                    g.wp_s = wp_pool.tile([P, g.KO, G, P], F16, tag="wp")

                    with (
                        tc.tile_pool(name="qh", bufs=2) as q_pool,
                        tc.tile_pool(name="kh", bufs=2) as k_pool,
                        tc.tile_pool(name="pt", bufs=8) as pt_pool,
                        tc.tile_pool(name="raccd", bufs=2) as raccd_pool,
                        tc.tile_pool(name="raccp", bufs=2) as raccp_pool,
                        tc.tile_pool(name="rsb", bufs=2) as rs_pool,
                        tc.tile_pool(name="rinv1", bufs=2) as rinv1_pool,
                        tc.tile_pool(name="rq", bufs=4) as rq_pool,
                        tc.tile_pool(name="dramq", bufs=4, space="DRAM")
                        as dramq,
                        tc.tile_pool(name="psS", bufs=2, space="PSUM") as psS,
                        tc.tile_pool(name="psO", bufs=2, space="PSUM") as psO,
                        tc.tile_pool(name="psN", bufs=2, space="PSUM") as psN,
                    ):
                        g.q_pool, g.k_pool, g.pt_pool = q_pool, k_pool, pt_pool
                        g.raccd_pool, g.raccp_pool = raccd_pool, raccp_pool
                        g.rs_pool, g.rinv1_pool = rs_pool, rinv1_pool
                        g.rq_pool, g.dramq = rq_pool, dramq
                        g.psS, g.psO, g.psN = psS, psO, psN
                        _phase_b(g)

                    with (
                        tc.tile_pool(name="csb", bufs=4) as csb_pool,
                        tc.tile_pool(name="psC", bufs=4, space="PSUM") as psC,
                    ):
                        g.csb_pool, g.psC = csb_pool, psC
                        _phase_c(g)

    _split_multi_waits(nc)
    return nc


# --------------------------------------------------------------------------
def _prep_core_inputs(xb, w_attn, w_proj, rope_cos, rope_sin, gidx, G=8,
                      n_half=2):
    """Host-side shard prep for one core: batch slice xb (T, C), group gidx."""
    T, C = xb.shape
    KO = C // P
    TH = T // n_half
    gc = gidx * G * P

    # x^T [H, p, kc, t]
    xt16 = np.ascontiguousarray(
        xb.T.reshape(KO, P, n_half, TH).transpose(2, 1, 0, 3)
    ).astype(np.float16)

    # q,k columns, RoPE pair-permuted (2i,2i+1) -> (i,64+i)
    perm = np.empty(P, dtype=np.int64)
    perm[:64] = np.arange(0, P, 2)
    perm[64:] = np.arange(1, P, 2)
    wq = w_attn[:, gc : gc + G * P].reshape(C, G, P)[:, :, perm]
    wk = w_attn[:, C + gc : C + gc + G * P].reshape(C, G, P)[:, :, perm]
    wqk_cols = np.concatenate(
        [wq.reshape(C, G * P), wk.reshape(C, G * P)], axis=1
    )
    wqk16 = np.ascontiguousarray(
        wqk_cols.reshape(KO, P, 2 * G, P).transpose(2, 1, 0, 3)
    ).astype(np.float16)

    wv_cols = w_attn[:, 2 * C + gc : 2 * C + gc + G * P]
    wv16 = np.ascontiguousarray(
        wv_cols.reshape(KO, P, G * P).transpose(1, 0, 2)
    ).astype(np.float16)

    wp_rows = w_proj[gc : gc + G * P, :]
    wp16 = np.ascontiguousarray(
        wp_rows.reshape(G, P, KO, P).transpose(2, 1, 0, 3)
    ).astype(np.float16)

    cT = rope_cos[:T].T
    sT = rope_sin[:T].T
    cosp = np.ascontiguousarray(np.concatenate([cT, cT], axis=0)).astype(
        np.float16
    )
    sinp = np.ascontiguousarray(np.concatenate([-sT, sT], axis=0)).astype(
        np.float16
    )
    maskz = np.concatenate(
        [np.zeros((P, 384), dtype=np.float32), np.triu(np.ones((P, P)))],
        axis=1,
    ).astype(np.float16)

    return {
        "xt16": xt16,
        "wqk16": wqk16,
        "wv16": wv16,
        "wp16": wp16,
        "cosp": cosp,
        "sinp": sinp,
        "maskz": maskz,
        "ones_c": np.ones((P, 1), dtype=np.float16),
        "ones_r": np.ones((1, P), dtype=np.float16),
    }


_NC_CACHE = {}
TRACE = False
LAST_RESULTS = None


def kernel(x, w_attn, w_proj, rope_cos, rope_sin):
    from concourse.bass_utils import run_bass_kernel_spmd

    x = np.asarray(x, dtype=np.float32)
    w_attn = np.asarray(w_attn, dtype=np.float32)
    w_proj = np.asarray(w_proj, dtype=np.float32)
    rope_cos = np.asarray(rope_cos, dtype=np.float32)
    rope_sin = np.asarray(rope_sin, dtype=np.float32)

    B, T, C = x.shape
    G = 8  # heads per group (16 heads / 2 groups)

    key = (T, C, G)
    if key not in _NC_CACHE:
        _NC_CACHE[key] = build_attention_core(T=T, C=C, G=G, n_half=2)
    nc = _NC_CACHE[key]

    in_maps = []
    for core in range(8):
        b, gi = core // 2, core % 2
        in_maps.append(
            _prep_core_inputs(x[b], w_attn, w_proj, rope_cos, rope_sin, gi,
                              G=G)
        )

    res = run_bass_kernel_spmd(nc, in_maps, list(range(8)), trace=TRACE)
    global LAST_RESULTS
    LAST_RESULTS = res

    y = np.empty((B, T, C), dtype=np.float32)
    for b in range(B):
        acc = res.results[2 * b]["outT"].astype(np.float32) + res.results[
            2 * b + 1
        ]["outT"].astype(np.float32)
        y[b] = acc.T
    return y
